# revision 1
# baseline (speedup 1.0000x reference)
"""Trainium2 Bass kernel for nn_CrossFusionModule_54485955117256.

Mathematical note driving the implementation
--------------------------------------------
The reference module ends with

    y  = fused @ Wb.T + bb                      # [B, S, 1]
    mu = mean(y, axis=-1, keepdims=True)        # axis has size 1  ->  mu == y
    var = mean((y - mu)**2, axis=-1)            # == 0 exactly
    yn = (y - mu) / sqrt(var + eps) * gamma + beta   # == beta exactly
    out = relu(yn)                              # == relu(beta), broadcast

The LayerNorm is taken over the last axis, which has size 1.  The mean of a
single element is that element bit-for-bit, so `y - mu == 0` exactly in
float32, `var == 0` exactly, and the normalized value collapses to `beta`
regardless of every preceding operation (projections, correlation matrix,
both softmax attentions, the bottleneck Linear).  All intermediates are
finite for any finite inputs, so no NaN/Inf can leak through the
cancellation.  The module's exact output is therefore

    out == relu(beta) broadcast to [B, S, 1]

independent of audio_feat / visual_feat and of every weight except `beta`.

The optimal kernel is thus a device-side ReLU of `beta` broadcast to the
output shape.  We keep the problem's data-parallel sharding: batch B=8 is
split across the 8 NeuronCores, each core computing its batch row's
[S, 1] = [2048, 1] output (laid out on-chip as a [128, 16] tile).

Per-core device program (3 instructions):
  1. DMA:    beta (replicated host-side to [128, 16], standard parameter
             replication) -> SBUF
  2. Vector: tensor_scalar_max(out, in, 0.0)  == ReLU, computed on device
  3. DMA:    SBUF -> DRAM output [128, 16]  (= the core's 2048 outputs)
"""

import sys

import numpy as np

for _p in ("/opt/trn_rl_repo", "/root/.axon_site/_ro/trn_rl_repo"):
    if _p not in sys.path:
        sys.path.append(_p)

# Problem constants (hardcoded from the module spec).
B = 8
S = 2048
N_CORES = 8
_P = 128                      # SBUF partitions
_F = S // _P                  # free-dim width per core: 2048/128 = 16

_NC_CACHE = {}


def _build_nc():
    """Build the per-core Bass program (identical SPMD program on 8 cores)."""
    import concourse.bass as bass
    import concourse.mybir as mybir

    nc = bass.Bass()
    beta_in = nc.declare_dram_parameter(
        "beta_rep", [_P, _F], mybir.dt.float32, isOutput=False
    )
    out = nc.declare_dram_parameter("out", [_P, _F], mybir.dt.float32, isOutput=True)

    with (
        nc.sbuf_tensor([_P, _F], mybir.dt.float32) as tin,
        nc.sbuf_tensor([_P, _F], mybir.dt.float32) as tout,
        nc.semaphore("dma_sem") as dma_sem,
        nc.semaphore("v_sem") as v_sem,
        nc.Block() as block,
    ):

        @block.sync
        def _(sync):
            sync.dma_start(out=tin[:, :], in_=beta_in[:, :]).then_inc(dma_sem, 16)
            sync.wait_ge(v_sem, 1)
            sync.dma_start(out=out[:, :], in_=tout[:, :]).then_inc(dma_sem, 16)
            sync.wait_ge(dma_sem, 32)

        @block.vector
        def _(vector):
            vector.wait_ge(dma_sem, 16)
            vector.tensor_scalar_max(tout[:, :], tin[:, :], 0.0).then_inc(v_sem, 1)

    return nc


def _get_nc():
    if "nc" not in _NC_CACHE:
        _NC_CACHE["nc"] = _build_nc()
    return _NC_CACHE["nc"]


def _run(inputs, trace=False, **spmd_kwargs):
    """Shard, run on 8 NeuronCores, gather.  Returns (output, BassKernelResults)."""
    from concourse.bass_utils import run_bass_kernel_spmd

    beta = np.asarray(inputs["beta"], dtype=np.float32).reshape(-1)[0]
    # Parameter replication (the module params are replicated across the
    # data-parallel cores); also laid out across the 128 SBUF partitions.
    beta_rep = np.full((_P, _F), beta, dtype=np.float32)

    nc = _get_nc()
    core_ids = list(range(N_CORES))
    in_maps = [{"beta_rep": beta_rep.copy()} for _ in core_ids]
    res = run_bass_kernel_spmd(nc, in_maps, core_ids, trace=trace, **spmd_kwargs)

    # Gather: core i produced batch row i's [S] outputs as a [128, 16] tile.
    out = np.stack(
        [np.asarray(res.results[i]["out"]).reshape(S, 1) for i in range(N_CORES)],
        axis=0,
    ).astype(np.float32)
    return out, res


def kernel(**inputs) -> np.ndarray:
    out, _ = _run(inputs)
    return out


# revision 2
# speedup vs baseline: 1.0968x; 1.0968x over previous
"""Trainium2 Bass kernel for nn_CrossFusionModule_54485955117256.

Mathematical note driving the implementation
--------------------------------------------
The reference module ends with

    y  = fused @ Wb.T + bb                      # [B, S, 1]
    mu = mean(y, axis=-1, keepdims=True)        # axis has size 1  ->  mu == y
    var = mean((y - mu)**2, axis=-1)            # == 0 exactly
    yn = (y - mu) / sqrt(var + eps) * gamma + beta   # == beta exactly
    out = relu(yn)                              # == relu(beta), broadcast

The LayerNorm is taken over the last axis, which has size 1.  The mean of a
single element is that element bit-for-bit, so `y - mu == 0` exactly in
float32, `var == 0` exactly, and the normalized value collapses to `beta`
regardless of every preceding operation (projections, correlation matrix,
both softmax attentions, the bottleneck Linear).  All intermediates are
finite for any finite inputs, so no NaN/Inf can leak through the
cancellation.  The module's exact output is therefore

    out == relu(beta) broadcast to [B, S, 1]

independent of audio_feat / visual_feat and of every weight except `beta`.

The optimal kernel is thus a device-side ReLU of `beta` broadcast to the
output shape.  We keep the problem's data-parallel sharding: batch B=8 is
split across the 8 NeuronCores, each core computing its batch row's
[S, 1] = [2048, 1] output (laid out on-chip as a [128, 16] tile).

Per-core device program (3 instructions):
  1. DMA:    beta (replicated host-side to [128, 16], standard parameter
             replication) -> SBUF
  2. Vector: tensor_scalar_max(out, in, 0.0)  == ReLU, computed on device
  3. DMA:    SBUF -> DRAM output [128, 16]  (= the core's 2048 outputs)
"""

import sys

import numpy as np

for _p in ("/opt/trn_rl_repo", "/root/.axon_site/_ro/trn_rl_repo"):
    if _p not in sys.path:
        sys.path.append(_p)

# Problem constants (hardcoded from the module spec).
B = 8
S = 2048
N_CORES = 8
_P = 128                      # SBUF partitions
_F = S // _P                  # free-dim width per core: 2048/128 = 16

_NC_CACHE = {}


def _build_nc():
    """Build the per-core Bass program (identical SPMD program on 8 cores)."""
    import concourse.bass as bass
    import concourse.mybir as mybir

    nc = bass.Bass()
    beta_in = nc.declare_dram_parameter(
        "beta_rep", [_P, _F], mybir.dt.float32, isOutput=False
    )
    out = nc.declare_dram_parameter("out", [_P, _F], mybir.dt.float32, isOutput=True)

    with (
        nc.sbuf_tensor([_P, _F], mybir.dt.float32) as tin,
        nc.sbuf_tensor([_P, _F], mybir.dt.float32) as tout,
        nc.semaphore("dma_sem") as dma_sem,
        nc.semaphore("v_sem") as v_sem,
        nc.Block(no_gpsimd_drain=True) as block,
    ):

        @block.sync
        def _(sync):
            sync.dma_start(out=tin[:, :], in_=beta_in[:, :]).then_inc(dma_sem, 16)
            sync.wait_ge(v_sem, 1)
            # No completion wait on the output DMA: the block-exit DRAIN on
            # the sync engine flushes its DMA queues before the NEFF ends.
            sync.dma_start(out=out[:, :], in_=tout[:, :]).then_inc(dma_sem, 16)

        @block.vector
        def _(vector):
            vector.wait_ge(dma_sem, 16)
            vector.tensor_scalar_max(tout[:, :], tin[:, :], 0.0).then_inc(v_sem, 1)

    return nc


def _get_nc():
    if "nc" not in _NC_CACHE:
        _NC_CACHE["nc"] = _build_nc()
    return _NC_CACHE["nc"]


def _run(inputs, trace=False, **spmd_kwargs):
    """Shard, run on 8 NeuronCores, gather.  Returns (output, BassKernelResults)."""
    from concourse.bass_utils import run_bass_kernel_spmd

    beta = np.asarray(inputs["beta"], dtype=np.float32).reshape(-1)[0]
    # Parameter replication (the module params are replicated across the
    # data-parallel cores); also laid out across the 128 SBUF partitions.
    beta_rep = np.full((_P, _F), beta, dtype=np.float32)

    nc = _get_nc()
    core_ids = list(range(N_CORES))
    in_maps = [{"beta_rep": beta_rep.copy()} for _ in core_ids]
    res = run_bass_kernel_spmd(nc, in_maps, core_ids, trace=trace, **spmd_kwargs)

    # Gather: core i produced batch row i's [S] outputs as a [128, 16] tile.
    out = np.stack(
        [np.asarray(res.results[i]["out"]).reshape(S, 1) for i in range(N_CORES)],
        axis=0,
    ).astype(np.float32)
    return out, res


def kernel(**inputs) -> np.ndarray:
    out, _ = _run(inputs)
    return out
